# revision 1
# baseline (speedup 1.0000x reference)
"""Memristive fully-connected layer on 8 Trainium2 NeuronCores.

The reference's differential conductance pair collapses algebraically:
g_pos - g_neg = g_eff = k_cond * weights, and the final rescale divides
K_V * k_cond back out, so the module computes exactly y = x @ w + b.

Strategy: data-parallel over the batch. Each core computes a
(1024 x 4096) @ (4096 x 4096) + b GEMM slice with float32r matmuls
(full-rate fp32 path on the PE array). The x shard is pre-transposed on
host so stationary-operand tiles are contiguous; the whole xT shard
(16.8 MB) stays resident in SBUF and w streams from HBM exactly once
per core. Bias is broadcast across partitions once and added on PSUM
eviction by the vector engine.

Per core: 8 n-blocks of 512 columns; the contraction runs in 8 k-blocks
of 4 k-tiles, sweeping all 8 output-row tiles per k-block, so each PSUM
bank's final matmul sits ~10 us ahead of the next block's first use and
evictions never stall the PE. DMAs are batched (4 k-tiles of w or xT
per transfer, 2 output tiles per store) to respect the 8 hardware DGE
queues, with w on the SP queue and xT/outputs on the Activation queue.
A short burst of throwaway matmuls during the initial DMA fill lifts
the PE's HAM clock gate before real work arrives.
"""

import numpy as np

import concourse.bass as bass  # noqa: F401  (registers engine classes)
import concourse.mybir as mybir
from concourse import bacc, tile
from concourse.bass_utils import run_bass_kernel_spmd

dt = mybir.dt

BATCH, N_IN, N_OUT = 8192, 4096, 4096
NCORES = 8
MB = BATCH // NCORES          # 1024 batch rows per core
P = 128
KT = N_IN // P                # 32 contraction tiles
MT = MB // P                  # 8 output-row tiles per core
NBLK = 512                    # matmul free dim (one PSUM bank)
NB = N_OUT // NBLK            # 8 output-column blocks
KB = 4                        # k-tiles per k-block (per w DMA)
NKB = KT // KB                # 8 k-blocks
WARMUP_MM = 26

_cache = {}


def _build():
    nc = bacc.Bacc("TRN2", target_bir_lowering=False, debug=False)
    xT = nc.dram_tensor("xT", [N_IN, MB], dt.float32r, kind="ExternalInput")
    w = nc.dram_tensor("w", [N_IN, N_OUT], dt.float32r, kind="ExternalInput")
    b = nc.dram_tensor("b", [1, N_OUT], dt.float32, kind="ExternalInput")
    y = nc.dram_tensor("y", [MB, N_OUT], dt.float32, kind="ExternalOutput")

    xT_r = xT.rearrange("(kt p) m -> p kt m", p=P)    # [128, 32, 1024]
    w_r = w.rearrange("(kt p) n -> p kt n", p=P)      # [128, 32, 4096]
    y_r = y.rearrange("(mt p) n -> p mt n", p=P)      # [128, 8, 4096]

    with tile.TileContext(nc) as tc:
        with (
            tc.tile_pool(name="xtp", bufs=1) as xtp,
            tc.tile_pool(name="wp", bufs=6) as wp,
            tc.tile_pool(name="bp", bufs=1) as bp,
            tc.tile_pool(name="op", bufs=3) as op,
            tc.tile_pool(name="ps", bufs=1, space="PSUM") as ps,
        ):
            # w k-block DMA, 4 k-tiles per transfer on the SP queue.
            # Returns the block as a list of per-k-tile [128, 512] views.
            def w_dma(nb, kb):
                wt = wp.tile([P, KB, NBLK], dt.float32r, name="wt")
                nc.sync.dma_start(
                    wt[:],
                    w_r[:, kb * KB:(kb + 1) * KB, nb * NBLK:(nb + 1) * NBLK],
                )
                return [wt[:, kk, :] for kk in range(KB)]

            xts = xtp.tile([P, KT, MB], dt.float32r, name="xts")

            def xt_dma(kb):
                nc.scalar.dma_start(
                    xts[:, kb * KB:(kb + 1) * KB, :],
                    xT_r[:, kb * KB:(kb + 1) * KB, :],
                )

            # HAM warmup: throwaway matmuls on a zeroed tile while the
            # first DMAs are in flight, so real matmuls start at 2.4 GHz.
            warm = bp.tile([P, 256], dt.float32, name="warm")
            nc.gpsimd.memset(warm[:], 0.0)
            wpsums = [
                ps.tile([P, NBLK], dt.float32, name=f"ps{i}") for i in range(MT)
            ]
            for i in range(WARMUP_MM):
                nc.tensor.matmul(
                    wpsums[i % MT][:, :256], warm[:, :P], warm[:],
                    start=True, stop=True,
                )

            # Startup DMAs in consumption order: the 8 hardware DGE queues
            # are assigned round-robin in emission order and each is FIFO,
            # so a soon-needed transfer must not sit behind a later-needed
            # one at a queue head. The first k-block's transfers are split
            # per k-tile so the first matmul's data lands in ~3 us instead
            # of waiting on multi-MB blocks.
            wts0 = [None] * NKB
            wts0[0] = w_dma(0, 0)
            xt_dma(0)
            wts0[1] = w_dma(0, 1)
            wts0[2] = w_dma(0, 2)
            xt_dma(1)
            wts0[3] = w_dma(0, 3)
            xt_dma(2)
            wts0[4] = w_dma(0, 4)
            xt_dma(3)
            wts0[5] = w_dma(0, 5)
            xt_dma(4)
            wts0[6] = w_dma(0, 6)
            xt_dma(5)
            wts0[7] = w_dma(0, 7)
            for kb in range(6, NKB):
                xt_dma(kb)

            # Bias: DMA the row into partition 0 of bias_sb, then broadcast
            # in place. Emitted after the warmup/startup DMAs — it rides the
            # slow gpsimd queue and is only needed at the first eviction
            # (~95 us in).
            bias_sb = bp.tile([P, N_OUT], dt.float32, name="bias_sb")
            nc.scalar.dma_start(bias_sb[0:1, :], b[:, :])
            nc.gpsimd.partition_broadcast(bias_sb[:], bias_sb[0:1, :])

            for nb in range(NB):
                psums = [
                    ps.tile([P, NBLK], dt.float32, name=f"ps{m}")
                    for m in range(MT)
                ]
                ot = None
                for kb in range(NKB):
                    if nb == 0:
                        wts = wts0[kb]
                    else:
                        wts = w_dma(nb, kb)
                    for m in range(MT):
                        for kk in range(KB):
                            k = kb * KB + kk
                            nc.tensor.matmul(
                                psums[m][:],
                                xts[:, k, m * P:(m + 1) * P],
                                wts[kk],
                                start=(k == 0),
                                stop=(k == KT - 1),
                            )
                        if kb == NKB - 1:
                            if nb == NB - 1:
                                # final block: single-tile stores alternated
                                # across both DMA queues shorten the tail
                                ot = op.tile([P, 2, NBLK], dt.float32, name="ot")
                                nc.vector.tensor_add(
                                    ot[:, 0, :],
                                    psums[m][:],
                                    bias_sb[:, nb * NBLK:(nb + 1) * NBLK],
                                )
                                eng = nc.scalar if m % 2 else nc.sync
                                eng.dma_start(
                                    y_r[:, m:m + 1, nb * NBLK:(nb + 1) * NBLK],
                                    ot[:, 0:1, :],
                                )
                            else:
                                if m % 2 == 0:
                                    ot = op.tile([P, 2, NBLK], dt.float32, name="ot")
                                nc.vector.tensor_add(
                                    ot[:, m % 2, :],
                                    psums[m][:],
                                    bias_sb[:, nb * NBLK:(nb + 1) * NBLK],
                                )
                                if m % 2 == 1:
                                    nc.scalar.dma_start(
                                        y_r[:, m - 1:m + 1, nb * NBLK:(nb + 1) * NBLK],
                                        ot[:],
                                    )
    nc.compile()
    return nc


def kernel(x, w, b, _trace=False, _trace_kwargs=None):
    if "nc" not in _cache:
        _cache["nc"] = _build()
    nc = _cache["nc"]

    b2 = np.ascontiguousarray(np.asarray(b, dtype=np.float32).reshape(1, N_OUT))
    w2 = np.ascontiguousarray(np.asarray(w, dtype=np.float32))
    in_maps = []
    for c in range(NCORES):
        xs = np.ascontiguousarray(x[c * MB:(c + 1) * MB].T.astype(np.float32))
        in_maps.append({"xT": xs, "w": w2, "b": b2})

    res = run_bass_kernel_spmd(
        nc,
        in_maps,
        core_ids=list(range(NCORES)),
        trace=_trace,
        **(_trace_kwargs or {}),
    )
    out = np.concatenate([res.results[c]["y"] for c in range(NCORES)], axis=0)
    if _trace:
        return out, res
    return out



# revision 2
# speedup vs baseline: 1.0633x; 1.0633x over previous
"""Memristive fully-connected layer on 8 Trainium2 NeuronCores.

The reference's differential conductance pair collapses algebraically:
g_pos - g_neg = g_eff = k_cond * weights, and the final rescale divides
K_V * k_cond back out, so the module computes exactly y = x @ w + b.

Strategy: data-parallel over the batch. Each core computes a
(1024 x 4096) @ (4096 x 4096) + b GEMM slice. Operands are cast to
fp16 on host (same PE rate as float32r, half the DMA/SBUF footprint,
FWL-eligible weight loads; rel err ~3e-4 vs the 2e-2 gate). The x
shard is pre-transposed on host; the whole xT shard (8.4 MB fp16)
stays resident in SBUF and w streams from HBM exactly once per core.
Bias is broadcast across partitions once and added on PSUM eviction by
the vector engine.

Per core: 8 n-blocks of 512 columns; the contraction runs in 8 k-blocks
of 4 k-tiles. Within a k-block the m loop is INNER so consecutive
matmuls rotate through all 8 PSUM banks (a same-bank back-to-back
matmul pays a write-port conflict). DMAs are batched (4 k-tiles of w or
xT per transfer, 2 output tiles per store). A short burst of throwaway
matmuls during the initial DMA fill lifts the PE's HAM clock gate
before real work arrives.
"""

import numpy as np

import concourse.bass as bass  # noqa: F401  (registers engine classes)
import concourse.mybir as mybir
from concourse import bacc, tile
from concourse.bass_utils import run_bass_kernel_spmd

dt = mybir.dt

BATCH, N_IN, N_OUT = 8192, 4096, 4096
NCORES = 8
MB = BATCH // NCORES          # 1024 batch rows per core
P = 128
KT = N_IN // P                # 32 contraction tiles
MT = MB // P                  # 8 output-row tiles per core
NBLK = 512                    # matmul free dim (one PSUM bank)
NB = N_OUT // NBLK            # 8 output-column blocks
KB = 4                        # k-tiles per k-block (per w DMA)
NKB = KT // KB                # 8 k-blocks
WARMUP_MM = 16

_cache = {}


def _build():
    nc = bacc.Bacc("TRN2", target_bir_lowering=False, debug=False)
    xT = nc.dram_tensor("xT", [N_IN, MB], dt.float16, kind="ExternalInput")
    w = nc.dram_tensor("w", [N_IN, N_OUT], dt.float16, kind="ExternalInput")
    b = nc.dram_tensor("b", [1, N_OUT], dt.float32, kind="ExternalInput")
    y = nc.dram_tensor("y", [MB, N_OUT], dt.float32, kind="ExternalOutput")

    xT_r = xT.rearrange("(kt p) m -> p kt m", p=P)    # [128, 32, 1024]
    w_r = w.rearrange("(kt p) n -> p kt n", p=P)      # [128, 32, 4096]
    y_r = y.rearrange("(mt p) n -> p mt n", p=P)      # [128, 8, 4096]

    with tile.TileContext(nc) as tc:
        with (
            tc.tile_pool(name="xtp", bufs=1) as xtp,
            tc.tile_pool(name="wp", bufs=8) as wp,
            tc.tile_pool(name="bp", bufs=1) as bp,
            tc.tile_pool(name="op", bufs=3) as op,
            tc.tile_pool(name="ps", bufs=1, space="PSUM") as ps,
        ):
            # w k-block DMA, 4 k-tiles per transfer on the SP queue.
            # Returns the block as a list of per-k-tile [128, 512] views.
            def w_dma(nb, kb):
                wt = wp.tile([P, KB, NBLK], dt.float16, name="wt")
                nc.sync.dma_start(
                    wt[:],
                    w_r[:, kb * KB:(kb + 1) * KB, nb * NBLK:(nb + 1) * NBLK],
                )
                return [wt[:, kk, :] for kk in range(KB)]

            xts = xtp.tile([P, KT, MB], dt.float16, name="xts")

            def xt_dma(kb):
                nc.scalar.dma_start(
                    xts[:, kb * KB:(kb + 1) * KB, :],
                    xT_r[:, kb * KB:(kb + 1) * KB, :],
                )

            # HAM warmup: throwaway matmuls on a zeroed tile while the
            # first DMAs are in flight, so real matmuls start at 2.4 GHz.
            warm = bp.tile([P, 256], dt.float16, name="warm")
            nc.vector.memset(warm[:], 0.0)
            wpsums = [
                ps.tile([P, NBLK], dt.float32, name=f"ps{i}") for i in range(MT)
            ]
            for i in range(WARMUP_MM):
                nc.tensor.matmul(
                    wpsums[i % MT][:, :256], warm[:, :P], warm[:],
                    start=True, stop=True,
                )

            # Startup DMAs in consumption order: the 8 hardware DGE queues
            # are assigned round-robin in emission order and each is FIFO,
            # so a soon-needed transfer must not sit behind a later-needed
            # one at a queue head.
            wts0 = [None] * NKB
            wts0[0] = w_dma(0, 0)
            xt_dma(0)
            wts0[1] = w_dma(0, 1)
            wts0[2] = w_dma(0, 2)
            xt_dma(1)
            wts0[3] = w_dma(0, 3)
            xt_dma(2)
            wts0[4] = w_dma(0, 4)
            xt_dma(3)
            wts0[5] = w_dma(0, 5)
            xt_dma(4)
            wts0[6] = w_dma(0, 6)
            xt_dma(5)
            wts0[7] = w_dma(0, 7)
            for kb in range(6, NKB):
                xt_dma(kb)

            # Bias: DMA the row into partition 0 of bias_sb, then broadcast
            # in place. Emitted after the warmup/startup DMAs — it rides the
            # slow gpsimd queue and is only needed at the first eviction.
            bias_sb = bp.tile([P, N_OUT], dt.float32, name="bias_sb")
            nc.scalar.dma_start(bias_sb[0:1, :], b[:, :])
            nc.gpsimd.partition_broadcast(bias_sb[:], bias_sb[0:1, :])

            for nb in range(NB):
                psums = [
                    ps.tile([P, NBLK], dt.float32, name=f"ps{m}")
                    for m in range(MT)
                ]
                ot = None
                for kb in range(NKB):
                    if nb == 0:
                        wts = wts0[kb]
                    else:
                        wts = w_dma(nb, kb)
                    for kk in range(KB):
                        k = kb * KB + kk
                        for m in range(MT):
                            nc.tensor.matmul(
                                psums[m][:],
                                xts[:, k, m * P:(m + 1) * P],
                                wts[kk],
                                start=(k == 0),
                                stop=(k == KT - 1),
                            )
                            if k != KT - 1:
                                continue
                            # final k-tile: evict this m's bank
                            if nb == NB - 1:
                                # final block: single-tile stores alternated
                                # across both DMA queues shorten the tail
                                ot = op.tile([P, 2, NBLK], dt.float32, name="ot")
                                nc.vector.tensor_add(
                                    ot[:, 0, :],
                                    psums[m][:],
                                    bias_sb[:, nb * NBLK:(nb + 1) * NBLK],
                                )
                                eng = nc.scalar if m % 2 else nc.sync
                                eng.dma_start(
                                    y_r[:, m:m + 1, nb * NBLK:(nb + 1) * NBLK],
                                    ot[:, 0:1, :],
                                )
                            else:
                                if m % 2 == 0:
                                    ot = op.tile([P, 2, NBLK], dt.float32, name="ot")
                                nc.vector.tensor_add(
                                    ot[:, m % 2, :],
                                    psums[m][:],
                                    bias_sb[:, nb * NBLK:(nb + 1) * NBLK],
                                )
                                if m % 2 == 1:
                                    nc.scalar.dma_start(
                                        y_r[:, m - 1:m + 1, nb * NBLK:(nb + 1) * NBLK],
                                        ot[:],
                                    )
    nc.compile()
    return nc


def kernel(x, w, b, _trace=False, _trace_kwargs=None):
    if "nc" not in _cache:
        _cache["nc"] = _build()
    nc = _cache["nc"]

    b2 = np.ascontiguousarray(np.asarray(b, dtype=np.float32).reshape(1, N_OUT))
    w2 = np.ascontiguousarray(np.asarray(w, dtype=np.float32).astype(np.float16))
    in_maps = []
    for c in range(NCORES):
        xs = np.ascontiguousarray(x[c * MB:(c + 1) * MB].T.astype(np.float16))
        in_maps.append({"xT": xs, "w": w2, "b": b2})

    res = run_bass_kernel_spmd(
        nc,
        in_maps,
        core_ids=list(range(NCORES)),
        trace=_trace,
        **(_trace_kwargs or {}),
    )
    out = np.concatenate([res.results[c]["y"] for c in range(NCORES)], axis=0)
    if _trace:
        return out, res
    return out


if __name__ == "__main__":
    rng = np.random.default_rng(0)
    x = rng.standard_normal((BATCH, N_IN), dtype=np.float32)
    w = rng.standard_normal((N_IN, N_OUT), dtype=np.float32) / np.sqrt(N_IN)
    b = rng.standard_normal((N_OUT,), dtype=np.float32) * 0.01
    y = kernel(x, w, b)
    ref = x @ w + b
    print("rel:", np.linalg.norm(y - ref) / np.linalg.norm(ref))


# revision 5
# speedup vs baseline: 1.0739x; 1.0099x over previous
"""Memristive fully-connected layer on 8 Trainium2 NeuronCores.

The reference's differential conductance pair collapses algebraically:
g_pos - g_neg = g_eff = k_cond * weights, and the final rescale divides
K_V * k_cond back out, so the module computes exactly y = x @ w + b.

Strategy: data-parallel over the batch. Each core computes a
(1024 x 4096) @ (4096 x 4096) + b GEMM slice. Operands are cast to
fp16 on host (same PE rate as float32r, half the DMA/SBUF footprint,
FWL-eligible weight loads; rel err ~3e-4 vs the 2e-2 gate) and
re-laid out partition-major so every DMA descriptor moves 4-8 KB of
contiguous HBM per partition. The whole xT shard (8.4 MB fp16) stays
resident in SBUF and w streams from HBM exactly once per core.

Per core: 8 n-blocks of 512 columns; the contraction runs in 8 k-blocks
of 4 k-tiles. Within a k-block the m loop is INNER so consecutive
matmuls rotate through all 8 PSUM banks (a same-bank back-to-back
matmul pays a write-port conflict; rotation keeps the steady cadence at
the 216 ns N=512 issue floor). w alternates between the SP and Pool
DGE queues, xT rides Activation, y stores ride Activation/SP. The
first k-block's transfers are split per k-tile so real matmuls start
early, and a warmup burst of throwaway matmuls keeps the PE's HAM
clock gate lifted until then. The final n-block runs as two half-m
passes so its PSUM drain overlaps compute instead of hanging off the
kernel tail.
"""

import numpy as np

import concourse.bass as bass  # noqa: F401  (registers engine classes)
import concourse.mybir as mybir
from concourse import bacc, tile
from concourse.bass_utils import run_bass_kernel_spmd

dt = mybir.dt

BATCH, N_IN, N_OUT = 8192, 4096, 4096
NCORES = 8
MB = BATCH // NCORES          # 1024 batch rows per core
P = 128
KT = N_IN // P                # 32 contraction tiles
MT = MB // P                  # 8 output-row tiles per core
NBLK = 512                    # matmul free dim (one PSUM bank)
NB = N_OUT // NBLK            # 8 output-column blocks
KB = 4                        # k-tiles per k-block (per w DMA)
NKB = KT // KB                # 8 k-blocks
WARMUP_MM = 28

_cache = {}


def _build():
    nc = bacc.Bacc("TRN2", target_bir_lowering=False, debug=False)
    # partition-major tiled layouts (see kernel() for the host shuffle):
    # xT2[p, kt, m]  = x_shard[m, kt*128 + p]
    # w2[p, nb, kt, n] = w[kt*128 + p, nb*512 + n]
    xT2 = nc.dram_tensor("xT2", [P, KT * MB], dt.float16, kind="ExternalInput")
    w2 = nc.dram_tensor("w2", [P, NB * KT * NBLK], dt.float16, kind="ExternalInput")
    b = nc.dram_tensor("b", [1, N_OUT], dt.float32, kind="ExternalInput")
    y = nc.dram_tensor("y", [MB, N_OUT], dt.float32, kind="ExternalOutput")

    xT_r = xT2.rearrange("p (kt m) -> p kt m", kt=KT)           # [128, 32, 1024]
    w_r = w2.rearrange("p (nb kt n) -> p nb kt n", nb=NB, kt=KT)  # [128, 8, 32, 512]
    y_r = y.rearrange("(mt p) n -> p mt n", p=P)                # [128, 8, 4096]

    with tile.TileContext(nc) as tc:
        with (
            tc.tile_pool(name="xtp", bufs=1) as xtp,
            tc.tile_pool(name="wp", bufs=10) as wp,
            tc.tile_pool(name="bp", bufs=1) as bp,
            tc.tile_pool(name="op", bufs=4) as op,
            tc.tile_pool(name="ps", bufs=1, space="PSUM") as ps,
        ):
            # w k-block DMA: 4 k-tiles per transfer (4 KB contiguous per
            # partition), alternating between the SP and Pool DGE queues.
            def w_dma(nb, kb, split=False):
                wt = wp.tile([P, KB, NBLK], dt.float16, name="wt")
                src = w_r[:, nb, kb * KB:(kb + 1) * KB, :]
                if split:
                    # per-k-tile pieces so the first matmul's data lands early
                    for kk in range(KB):
                        e = (nc.sync, nc.gpsimd)[kk % 2]
                        e.dma_start(wt[:, kk:kk + 1, :], src[:, kk:kk + 1, :])
                else:
                    eng = nc.sync if (nb * NKB + kb) % 2 == 0 else nc.gpsimd
                    eng.dma_start(wt[:], src)
                return [wt[:, kk, :] for kk in range(KB)]

            xts = xtp.tile([P, KT, MB], dt.float16, name="xts")

            def xt_dma(kb, split=False):
                dst = xts[:, kb * KB:(kb + 1) * KB, :]
                src = xT_r[:, kb * KB:(kb + 1) * KB, :]
                if split:
                    for kk in range(KB):
                        e = (nc.scalar, nc.gpsimd)[kk % 2]
                        e.dma_start(dst[:, kk:kk + 1, :], src[:, kk:kk + 1, :])
                else:
                    nc.scalar.dma_start(dst, src)

            # HAM warmup: throwaway matmuls on a zeroed tile while the
            # first DMAs are in flight, so real matmuls start at 2.4 GHz.
            warm = bp.tile([P, 256], dt.float16, name="warm")
            nc.vector.memset(warm[:], 0.0)
            wpsums = [
                ps.tile([P, NBLK], dt.float32, name=f"ps{i}") for i in range(MT)
            ]
            for i in range(WARMUP_MM):
                nc.tensor.matmul(
                    wpsums[i % MT][:, :256], warm[:, :P], warm[:],
                    start=True, stop=True,
                )

            # Startup DMAs in consumption order; first k-block split fine.
            wts0 = [None] * NKB
            wts0[0] = w_dma(0, 0, split=True)
            xt_dma(0, split=True)
            wts0[1] = w_dma(0, 1)
            wts0[2] = w_dma(0, 2)
            xt_dma(1)
            wts0[3] = w_dma(0, 3)
            xt_dma(2)
            wts0[4] = w_dma(0, 4)
            xt_dma(3)
            wts0[5] = w_dma(0, 5)
            xt_dma(4)
            wts0[6] = w_dma(0, 6)
            xt_dma(5)
            wts0[7] = w_dma(0, 7)
            xt_dma(6)
            xt_dma(7)

            # Bias: DMA the row into partition 0 of bias_sb, then broadcast
            # in place; only needed at the first eviction (~60 us in).
            bias_sb = bp.tile([P, N_OUT], dt.float32, name="bias_sb")
            nc.gpsimd.dma_start(bias_sb[0:1, :], b[:, :])
            nc.gpsimd.partition_broadcast(bias_sb[:], bias_sb[0:1, :])

            for nb in range(NB):
                psums = [
                    ps.tile([P, NBLK], dt.float32, name=f"ps{m}")
                    for m in range(MT)
                ]

                def evict(m, ot, slot, nb=nb):
                    nc.vector.tensor_add(
                        ot[:, slot, :],
                        psums[m][:],
                        bias_sb[:, nb * NBLK:(nb + 1) * NBLK],
                    )

                final = nb == NB - 1
                # Final block: two half-m passes so the first half's PSUM
                # drain overlaps the second half's matmuls.
                m_passes = ([range(0, 4), range(4, 8)] if final else [range(MT)])
                ot = None
                wts_by_kb = {}
                for mp, m_range in enumerate(m_passes):
                    for kb in range(NKB):
                        if mp == 0:
                            wts_by_kb[kb] = wts0[kb] if nb == 0 else w_dma(nb, kb)
                        wts = wts_by_kb[kb]
                        for kk in range(KB):
                            k = kb * KB + kk
                            for m in m_range:
                                nc.tensor.matmul(
                                    psums[m][:],
                                    xts[:, k, m * P:(m + 1) * P],
                                    wts[kk],
                                    start=(k == 0),
                                    stop=(k == KT - 1),
                                )
                                if k != KT - 1:
                                    continue
                                if final:
                                    # single-tile stores spread over the DGE
                                    # queues to shorten the drain
                                    ot = op.tile([P, 2, NBLK], dt.float32, name="ot")
                                    evict(m, ot, 0)
                                    eng = (nc.scalar, nc.sync, nc.gpsimd)[m % 3]
                                    eng.dma_start(
                                        y_r[:, m:m + 1, nb * NBLK:(nb + 1) * NBLK],
                                        ot[:, 0:1, :],
                                    )
                                else:
                                    if m % 2 == 0:
                                        ot = op.tile([P, 2, NBLK], dt.float32, name="ot")
                                    evict(m, ot, m % 2)
                                    if m % 2 == 1:
                                        eng = nc.scalar if (nb + m) % 4 == 1 else nc.sync
                                        eng.dma_start(
                                            y_r[:, m - 1:m + 1, nb * NBLK:(nb + 1) * NBLK],
                                            ot[:],
                                        )
    nc.compile()
    return nc


def kernel(x, w, b, _trace=False, _trace_kwargs=None):
    if "nc" not in _cache:
        _cache["nc"] = _build()
    nc = _cache["nc"]

    b2 = np.ascontiguousarray(np.asarray(b, dtype=np.float32).reshape(1, N_OUT))
    # w2[p, nb, kt, n] = w[kt*128 + p, nb*512 + n]
    w16 = np.asarray(w, dtype=np.float32).astype(np.float16)
    w2 = np.ascontiguousarray(
        w16.reshape(KT, P, NB, NBLK).transpose(1, 2, 0, 3).reshape(P, -1)
    )
    in_maps = []
    for c in range(NCORES):
        xs = np.asarray(x[c * MB:(c + 1) * MB], dtype=np.float32).astype(np.float16)
        # xT2[p, kt, m] = x_shard[m, kt*128 + p]
        xT2 = np.ascontiguousarray(
            xs.T.reshape(KT, P, MB).transpose(1, 0, 2).reshape(P, -1)
        )
        in_maps.append({"xT2": xT2, "w2": w2, "b": b2})

    res = run_bass_kernel_spmd(
        nc,
        in_maps,
        core_ids=list(range(NCORES)),
        trace=_trace,
        **(_trace_kwargs or {}),
    )
    out = np.concatenate([res.results[c]["y"] for c in range(NCORES)], axis=0)
    if _trace:
        return out, res
    return out


if __name__ == "__main__":
    rng = np.random.default_rng(0)
    x = rng.standard_normal((BATCH, N_IN), dtype=np.float32)
    w = rng.standard_normal((N_IN, N_OUT), dtype=np.float32) / np.sqrt(N_IN)
    b = rng.standard_normal((N_OUT,), dtype=np.float32) * 0.01
    y = kernel(x, w, b)
    ref = x @ w + b
    print("rel:", np.linalg.norm(y - ref) / np.linalg.norm(ref))
